# revision 23
# baseline (speedup 1.0000x reference)
"""Trainium2 Bass kernel for nn_Atten_50611894616900.

Math note: in the reference, softmax is applied over a trailing singleton
dimension, so the attention weights are exactly 1.0.  The whole network
therefore reduces to

    y   = (h.sum(axis=1)) @ (Wv @ Wo) + bo          # [n, 128]
    out = relu(batchnorm(y))                        # batch stats over n

and since batchnorm subtracts the per-channel mean, the per-channel bias
`bo` cancels exactly (it also leaves the variance unchanged).  `x`, `Wq`,
`Wk` and `bo` do not influence the output at all.

Strategy: data-parallel over points across 8 NeuronCores (4096 points per
core).  Per core we stream `h` (32 MB) once, reduce over the 16 neighbors
with identity-stationary accumulating matmuls on the TensorEngine, project
through the fused [128,128] weight W = Wv@Wo (computed on device), keep the
activations resident in SBUF transposed as yT [channel, point], compute
BatchNorm statistics with bn_stats/bn_aggr, AllReduce the (mean, E[x^2])
pair across the 8 cores (1 KB), then apply relu(yT*A + B) with per-channel
scale/bias in a single scalar-engine op per tile, transpose back with the
TensorEngine and DMA out.
"""

import numpy as np

import concourse.bass as bass
import concourse.bacc as bacc
import concourse.tile as tile
import concourse.mybir as mybir
from concourse.masks import make_identity
from concourse.bass_utils import run_bass_kernel_spmd

N_POINTS = 32768
N_CORES = 8
CH = 128            # input/output channels
KNB = 16            # neighbors per point
FREE = KNB * CH     # 2048 = flattened (k, c) extent of h per point
INNER = 512
BN_EPS = 1e-5
P = 128             # partition count / point-block size
CHUNK_BLKS = 8      # 1024 points (4 MB bf16) per input DMA

F32 = mybir.dt.float32
BF16 = mybir.dt.bfloat16
AF = mybir.ActivationFunctionType
H_BF16 = True       # stream h as bf16 (halves DMA; rel err ~1.7e-3)


def build_body(tc, out_d, h_d, wv_d, wo_d, gam_d, bet_d, n_loc, n_cores,
               n_total, reps=1, chunk_blks=CHUNK_BLKS, hbufs=4,
               use_collective=True, h_bf16=H_BF16):
    """Emit the per-core program. h_d is [n_loc, 2048], out_d is [n_loc, 128].

    gam_d / bet_d are [128, 1] so they land with channel on the partition
    axis directly.
    """
    nc = tc.nc
    nblk = n_loc // P
    nchunks = (nblk + chunk_blks - 1) // chunk_blks

    with (
        tc.tile_pool(name="singles", bufs=1) as singles,
        tc.tile_pool(name="hpool", bufs=hbufs) as hpool,
        tc.tile_pool(name="small", bufs=3) as small,
        tc.tile_pool(name="opool", bufs=4) as opool,
        tc.tile_pool(name="pacc", bufs=2, space="PSUM") as pacc,
        tc.tile_pool(name="ppt", bufs=2, space="PSUM") as ppt,
        tc.tile_pool(name="ppy", bufs=2, space="PSUM") as ppy,
        tc.tile_pool(name="dpool", bufs=1, space="DRAM") as dpool,
    ):
        ident = singles.tile([P, P], F32)
        make_identity(nc, ident)
        h_dt = BF16 if h_bf16 else F32
        if h_bf16:
            ident_h = singles.tile([P, P], BF16)
            make_identity(nc, ident_h)
        else:
            ident_h = ident

        # h chunk DMA plan: small chunks first (fast pipeline start) and
        # last (short serial chain after the final byte lands), big 4MB
        # chunks in the middle. All sizes are multiples of GRP except
        # possibly the last, so 512-point compute groups never straddle
        # chunks.
        GRP = 4
        ngrp = (nblk + GRP - 1) // GRP
        plan = []
        rem = nblk
        while rem > 0:
            b = min(chunk_blks, rem)
            plan.append(b)
            rem -= b
        starts = [0]
        for b in plan[:-1]:
            starts.append(starts[-1] + b)
        blk2chunk = {}
        for ci, (s, b) in enumerate(zip(starts, plan)):
            for blk in range(s, s + b):
                blk2chunk[blk] = ci
        nchunks = len(plan)
        pending = {}

        def issue_chunk(j):
            blks = plan[j]
            h_tile = hpool.tile([P, chunk_blks, FREE], h_dt, tag="h",
                                name=f"h_tile_{j}")
            n0 = starts[j] * P
            nc.sync.dma_start(
                out=h_tile[:, :blks, :],
                in_=h_d[n0:n0 + blks * P, :].rearrange("(b p) q -> p b q",
                                                       p=P),
            )
            return h_tile

        pending[0] = issue_chunk(0)

        # ---- fused projection weight W = Wv @ Wo, [CH, CH] ----
        wv_s = singles.tile([CH, INNER], F32)
        nc.sync.dma_start(out=wv_s, in_=wv_d)
        wo_s = singles.tile([P, 4, CH], F32)
        nc.sync.dma_start(out=wo_s, in_=wo_d.rearrange("(q i) c -> i q c", i=P))
        wvt_s = singles.tile([P, 4, CH], F32)
        for q in range(4):
            tp = ppt.tile([P, CH], F32, tag="pt")
            nc.tensor.matmul(tp, lhsT=wv_s[:, q * P:(q + 1) * P], rhs=ident,
                             start=True, stop=True)
            nc.scalar.copy(out=wvt_s[:, q, :], in_=tp)
        w_ps = ppy.tile([CH, CH], F32, tag="py")
        for q in range(4):
            nc.tensor.matmul(w_ps, lhsT=wvt_s[:, q, :], rhs=wo_s[:, q, :],
                             start=(q == 0), stop=(q == 3))
        w_s = singles.tile([CH, CH], F32)
        nc.scalar.copy(out=w_s, in_=w_ps)
        w_h = singles.tile([CH, CH], h_dt, name="w_h")
        nc.scalar.copy(out=w_h, in_=w_s)

        gam_s = singles.tile([CH, 1], F32)
        nc.sync.dma_start(out=gam_s, in_=gam_d)
        bet_s = singles.tile([CH, 1], F32)
        nc.sync.dma_start(out=bet_s, in_=bet_d)
        eps_s = singles.tile([CH, 1], F32)
        nc.vector.memset(eps_s, BN_EPS)
        warm = singles.tile([1, 1], F32)
        nc.scalar.activation(out=warm, in_=eps_s[0:1, 0:1], func=AF.Sqrt)

        # activations stay resident, transposed: yT[channel, point]
        y_store = singles.tile([CH, n_loc], F32)
        stats = singles.tile([CH, ngrp, 6], F32)

        # ---- main streaming pass over h ----
        def stream_pass():
            cur = {}
            for g in range(ngrp):
                gb0 = g * GRP
                gblks = min(GRP, nblk - gb0)
                gw = gblks * P
                n0 = gb0 * P
                j = blk2chunk[gb0]
                if j not in cur:
                    cur.clear()
                    cur[j] = pending.pop(j) if j in pending else \
                        issue_chunk(j)
                    if j + 1 < nchunks and j + 1 not in pending:
                        pending[j + 1] = issue_chunk(j + 1)
                h_tile = cur[j]
                boff = gb0 - starts[j]
                # hsum[pt, b, c] = sum_k h[pt, b, k, c]: identity-stationary
                # accumulating matmuls, free dim = (b, c) = gw
                ps_hsum = pacc.tile([P, GRP, CH], F32, tag="hs")
                for k in range(KNB):
                    nc.tensor.matmul(
                        ps_hsum[:, :gblks, :], lhsT=ident_h,
                        rhs=h_tile[:, boff:boff + gblks,
                                   k * CH:(k + 1) * CH],
                        start=(k == 0), stop=(k == KNB - 1))
                hsum_s = small.tile([P, GRP, CH], h_dt, tag="hsum")
                nc.scalar.copy(out=hsum_s[:, :gblks, :],
                               in_=ps_hsum[:, :gblks, :])
                # transpose each 128-pt block: hT[c, b, pt] = hsum[pt, b, c].T
                # (PE transpose datapath: 2 cyc/row for f32 vs 4 normal)
                ps_ht = ppt.tile([CH, GRP, P], h_dt, tag="pth")
                for b in range(gblks):
                    nc.tensor.matmul(ps_ht[:, b, :], lhsT=hsum_s[:, b, :],
                                     rhs=ident_h, is_transpose=True,
                                     start=True, stop=True)
                ht_s = small.tile([CH, GRP, P], h_dt, tag="ht")
                nc.scalar.copy(out=ht_s[:, :gblks, :],
                               in_=ps_ht[:, :gblks, :])
                # yT[c_out, (b, pt)] = sum_c W[c, c_out] * hT[c, (b, pt)]
                ps_yt = ppy.tile([CH, GRP, P], F32, tag="py")
                nc.tensor.matmul(ps_yt[:, :gblks, :], lhsT=w_h,
                                 rhs=ht_s[:, :gblks, :],
                                 start=True, stop=True)
                nc.scalar.copy(out=y_store[:, n0:n0 + gw],
                               in_=ps_yt[:, :gblks, :])
                nc.vector.bn_stats(
                    out=stats[:, g, :],
                    in_=ps_yt[:, :gblks, :].rearrange("c b p -> c (b p)"))

        # (reps>1 re-runs the pass for timing amplification only)
        for _rep in range(reps):
            stream_pass()

        # ---- local batch stats -> (mean, E[x^2]) for the all-reduce ----
        mv = singles.tile([CH, 2], F32)
        nc.vector.bn_aggr(out=mv, in_=stats)
        musq = singles.tile([CH, 1], F32)
        nc.vector.tensor_mul(musq, mv[:, 0:1], mv[:, 0:1])
        arbuf = singles.tile([CH, 2], F32)
        nc.vector.tensor_copy(arbuf[:, 0:1], mv[:, 0:1])
        nc.vector.tensor_add(arbuf[:, 1:2], mv[:, 1:2], musq)

        cc_in = dpool.tile([CH, 2], F32)
        cc_out = dpool.tile([CH, 2], F32, addr_space="Shared")
        nc.sync.dma_start(out=cc_in, in_=arbuf)
        if use_collective:
            nc.gpsimd.collective_compute(
                "AllReduce",
                mybir.AluOpType.add,
                replica_groups=[list(range(n_cores))],
                ins=[cc_in.opt()],
                outs=[cc_out.opt()],
            )
        else:
            nc.sync.dma_start(out=cc_out, in_=cc_in)
        ar_s = singles.tile([CH, 2], F32)
        nc.sync.dma_start(out=ar_s, in_=cc_out)

        # ---- global mean/var -> per-channel scale A and shift B ----
        invc = 1.0 / n_cores
        mu = singles.tile([CH, 1], F32)
        nc.scalar.mul(out=mu, in_=ar_s[:, 0:1], mul=invc)
        ex2 = singles.tile([CH, 1], F32)
        nc.scalar.mul(out=ex2, in_=ar_s[:, 1:2], mul=invc)
        var = singles.tile([CH, 1], F32)
        nc.vector.tensor_mul(var, mu, mu)
        nc.vector.tensor_sub(var, ex2, var)
        std = singles.tile([CH, 1], F32)
        nc.scalar.activation(out=std, in_=var, func=AF.Sqrt,
                             bias=eps_s, scale=1.0)
        inv = singles.tile([CH, 1], F32)
        nc.vector.reciprocal(out=inv, in_=std)
        a_vec = singles.tile([CH, 1], F32)
        nc.vector.tensor_mul(a_vec, gam_s, inv)
        b_vec = singles.tile([CH, 1], F32)
        nc.vector.tensor_mul(b_vec, mu, a_vec)
        nc.vector.tensor_sub(b_vec, bet_s, b_vec)

        # ---- normalize + relu in yT (per-channel = per-partition affine),
        # then transpose back per chunk and store with one batched DMA ----
        yn = singles.tile([CH, n_loc], F32)
        for g in range(ngrp):
            gblks = min(GRP, nblk - g * GRP)
            gw = gblks * P
            n0 = g * GRP * P
            # z = y*A + B on DVE (per-partition scalars); relu rides the
            # post-transpose PSUM->SBUF copy on ACT (relu commutes with
            # the permutation).
            nc.vector.tensor_scalar(out=yn[:, n0:n0 + gw],
                                    in0=y_store[:, n0:n0 + gw],
                                    scalar1=a_vec, scalar2=b_vec,
                                    op0=mybir.AluOpType.mult,
                                    op1=mybir.AluOpType.add)
            ps_o = ppt.tile([P, GRP, CH], F32, tag="pt")
            for b in range(gblks):
                nc.tensor.matmul(ps_o[:, b, :],
                                 lhsT=yn[:, n0 + b * P:n0 + (b + 1) * P],
                                 rhs=ident, is_transpose=True,
                                 start=True, stop=True)
            o_s = opool.tile([P, GRP, CH], F32, tag="o")
            nc.scalar.activation(out=o_s[:, :gblks, :],
                                 in_=ps_o[:, :gblks, :], func=AF.Relu)
            nc.sync.dma_start(
                out=out_d[n0:n0 + gw, :].rearrange("(b p) c -> p b c", p=P),
                in_=o_s[:, :gblks, :])


def build(n_loc, n_cores, n_total, reps=1, chunk_blks=CHUNK_BLKS, hbufs=4,
          use_collective=True, h_bf16=H_BF16):
    nc = bacc.Bacc("TRN2", target_bir_lowering=False, debug=False,
                   num_devices=n_cores)
    h_d = nc.dram_tensor("h", [n_loc, FREE], BF16 if h_bf16 else F32,
                         kind="ExternalInput").ap()
    wv_d = nc.dram_tensor("Wv", [CH, INNER], F32, kind="ExternalInput").ap()
    wo_d = nc.dram_tensor("Wo", [INNER, CH], F32, kind="ExternalInput").ap()
    gam_d = nc.dram_tensor("gamma", [CH, 1], F32, kind="ExternalInput").ap()
    bet_d = nc.dram_tensor("beta", [CH, 1], F32, kind="ExternalInput").ap()
    out_d = nc.dram_tensor("out", [n_loc, CH], F32, kind="ExternalOutput").ap()
    with tile.TileContext(nc) as tc:
        build_body(tc, out_d, h_d, wv_d, wo_d, gam_d, bet_d,
                   n_loc, n_cores, n_total, reps=reps,
                   chunk_blks=chunk_blks, hbufs=hbufs,
                   use_collective=use_collective, h_bf16=h_bf16)
    nc.compile()
    return nc


_NC_CACHE = {}


def _get_nc(n_loc, n_cores, n_total):
    key = (n_loc, n_cores, n_total)
    if key not in _NC_CACHE:
        _NC_CACHE[key] = build(n_loc, n_cores, n_total)
    return _NC_CACHE[key]


def _run(inputs, trace=False):
    import ml_dtypes
    h = np.asarray(inputs["h"])
    if H_BF16:
        h = np.ascontiguousarray(h.astype(ml_dtypes.bfloat16))
    else:
        h = np.ascontiguousarray(h.astype(np.float32))
    wv = np.ascontiguousarray(np.asarray(inputs["Wv"], dtype=np.float32))
    wo = np.ascontiguousarray(np.asarray(inputs["Wo"], dtype=np.float32))
    gamma = np.ascontiguousarray(
        np.asarray(inputs["gamma"], dtype=np.float32).reshape(CH, 1))
    beta = np.ascontiguousarray(
        np.asarray(inputs["beta"], dtype=np.float32).reshape(CH, 1))

    n = h.shape[0]
    assert n % N_CORES == 0
    n_loc = n // N_CORES
    hf = h.reshape(n, FREE)

    nc = _get_nc(n_loc, N_CORES, n)
    in_maps = [
        {
            "h": np.ascontiguousarray(hf[c * n_loc:(c + 1) * n_loc]),
            "Wv": wv,
            "Wo": wo,
            "gamma": gamma,
            "beta": beta,
        }
        for c in range(N_CORES)
    ]
    res = run_bass_kernel_spmd(nc, in_maps, core_ids=list(range(N_CORES)),
                               trace=trace)
    out = np.concatenate([r["out"] for r in res.results], axis=0)
    return out, res


def kernel(**inputs):
    out, _ = _run(inputs, trace=False)
    return out
